# revision 1
# baseline (speedup 1.0000x reference)
"""Distributed Bass kernel for causal multi-head attention with RoPE on 8 TRN2 NeuronCores.

Problem (hardcoded): x [2, 2048, 1024] f32, wQKV [3072, 1024], wOut [1024, 1024],
cos/sin [2048, 32]; 16 heads, dh=64; out = causal-attention(x) @ wOut.T.

Sharding: 8 cores = 2 (batch) x 4 (head-group). Core (b, g) computes heads
4g..4g+3 of batch b. After attention, an AllGather within each 4-core batch
group collects all heads' attn output (transposed layout [dims, seq]); each
core then applies its 256-column slice of wOut, producing outT [256, 2048].
Host reassembles [2, 2048, 1024].

Compute dtype: bf16 matmuls with f32 PSUM accumulation (rel err ~1e-3 vs f32).

Device layout notes:
- All matmul operands keep the contraction dim on partitions, so x, wQKV and
  wOut are pre-transposed on the host (pure layout prep, no arithmetic).
- RoPE is applied in transposed layout. wQKV rows are pre-permuted per head
  group as [qE(128), qO(128), kE(128), kO(128), v(256)] where E/O are the
  even/odd head-dim components of all 4 local heads; RoPE is then 12
  full-width [128, 2048] vector ops, after which q/k are repacked into
  per-head-contiguous tiles for the attention matmuls.
- Scores are computed transposed (sT[kr, qr] = k.T q) so the exp output feeds
  the PV matmul directly as the moving operand, with a ones-column appended to
  v as lhsT so the softmax denominators fall out of the same matmul (row 64).
"""

import numpy as np
import ml_dtypes

BF = ml_dtypes.bfloat16

B, S, D, H = 2, 2048, 1024, 16
DH = 64          # head dim
NF = 32          # rope freqs = DH/2
HL = 4           # heads per core
QB = 512         # qr block width
KC = 128         # kr chunk
NC_ = 8          # cores
RG = [[0, 1, 2, 3], [4, 5, 6, 7]]

_cached = {}


def _build_nc(nrep=1, loop_iters=None, collectives=True):
    import concourse.bass as bass
    import concourse.bacc as bacc
    import concourse.mybir as mybir
    import concourse.tile as tile
    from concourse import masks

    FP32 = mybir.dt.float32
    BF16 = mybir.dt.bfloat16
    Exp = mybir.ActivationFunctionType.Exp

    nc = bacc.Bacc(
        "TRN2", target_bir_lowering=False, debug=False, num_devices=NC_
    )

    xT_e = nc.dram_tensor("xT", [D, S], BF16, kind="ExternalInput")
    wqkvT_e = nc.dram_tensor("wqkvT", [D, 768], BF16, kind="ExternalInput")
    woutT_e = nc.dram_tensor("woutT", [D, 256], BF16, kind="ExternalInput")
    cosT_e = nc.dram_tensor("cosT", [128, S], BF16, kind="ExternalInput")
    sinT_e = nc.dram_tensor("sinT", [128, S], BF16, kind="ExternalInput")
    dmask_e = nc.dram_tensor("dmask", [128, 4 * QB], BF16, kind="ExternalInput")
    # output split into 256KB chunks: big single d2h transfers hang through
    # the axon tunnel. out{j} covers s columns [256j, 256j+256).
    out_e = [
        nc.dram_tensor(f"out{j}", [256, 256], FP32, kind="ExternalOutput")
        for j in range(8)
    ]

    with tile.TileContext(nc) as tc:
        with (
            tc.tile_pool(name="pconst", bufs=1) as pconst,
            tc.tile_pool(name="pw", bufs=1) as pw,
            tc.tile_pool(name="px", bufs=1) as px,
            tc.tile_pool(name="pqkv", bufs=1) as pqkv,
            tc.tile_pool(name="ptmp", bufs=3) as ptmp,
            tc.tile_pool(name="pwt", bufs=8) as pwt,
            tc.tile_pool(name="pao", bufs=4) as pao,
            tc.tile_pool(name="pagg", bufs=2) as pagg,
            tc.tile_pool(name="pout", bufs=2) as pout,
            tc.tile_pool(name="psA", bufs=2, space="PSUM") as psA,
            tc.tile_pool(name="psB", bufs=2, space="PSUM") as psB,
            tc.tile_pool(name="psAT", bufs=2, space="PSUM") as psAT,
            tc.tile_pool(name="pdram", bufs=1, space="DRAM") as pdram,
        ):
            # ---- constants ----
            cos_sb = pconst.tile([128, S], BF16, tag="cos")
            sin_sb = pconst.tile([128, S], BF16, tag="sin")
            dmask_sb = pconst.tile([128, 4 * QB], BF16, tag="dmask")
            ident_sb = pconst.tile([128, 128], BF16, tag="ident")
            ones_sb = pconst.tile([1, 64], FP32, tag="ones")
            nc.sync.dma_start(cos_sb[:], cosT_e[:, :])
            nc.sync.dma_start(sin_sb[:], sinT_e[:, :])
            nc.sync.dma_start(dmask_sb[:], dmask_e[:, :])
            masks.make_identity(nc, ident_sb[:])
            nc.vector.memset(ones_sb[:], 1.0)

            # ---- weights + x ----
            wqkv_sb = []
            wout_sb = []
            x_sb = []
            for i in range(8):
                w = pw.tile([128, 768], BF16, tag=f"wqkv{i}", name=f"wqkv{i}")
                nc.sync.dma_start(w[:], wqkvT_e[128 * i:128 * (i + 1), :])
                wqkv_sb.append(w)
                wo = pw.tile([128, 256], BF16, tag=f"wout{i}", name=f"wout{i}")
                nc.sync.dma_start(wo[:], woutT_e[128 * i:128 * (i + 1), :])
                wout_sb.append(wo)
                xt = px.tile([128, S], BF16, tag=f"x{i}", name=f"x{i}")
                nc.sync.dma_start(xt[:], xT_e[128 * i:128 * (i + 1), :])
                x_sb.append(xt)

            import contextlib

            def _rep_scope():
                if loop_iters is not None:
                    return tc.For_i(0, loop_iters, 1)
                return contextlib.nullcontext()

            for rep in range(nrep):
              with _rep_scope():
                  # ---- QKV projection: qkvT[od, s] over 6 od-tiles ----
                  # od-tiles: 0 qE, 1 qO, 2 kE, 3 kO, 4 v(first 128), 5 v(last 128)
                  qkv_t = [pqkv.tile([128, S], BF16, tag=f"qkv{i}", name=f"qkv{i}") for i in range(6)]
                  for ot in range(6):
                      for sp in range(2):
                          ps = psB.tile([128, 2 * QB], FP32, tag="mmB")
                          for half in range(2):
                              sc_i = 2 * sp + half
                              for dc in range(8):
                                  nc.tensor.matmul(
                                      ps[:, QB * half:QB * (half + 1)],
                                      lhsT=wqkv_sb[dc][:, 128 * ot:128 * (ot + 1)],
                                      rhs=x_sb[dc][:, QB * sc_i:QB * (sc_i + 1)],
                                      start=(dc == 0),
                                      stop=(dc == 7),
                                  )
                          nc.vector.tensor_copy(
                              qkv_t[ot][:, 2 * QB * sp:2 * QB * (sp + 1)], ps[:]
                          )

                  qE, qO, kE, kO, v0, v1 = qkv_t

                  # ---- RoPE (full-width ops in E/O layout) ----
                  qE2 = pqkv.tile([128, S], BF16, tag="qE2")
                  qO2 = pqkv.tile([128, S], BF16, tag="qO2")
                  kE2 = pqkv.tile([128, S], BF16, tag="kE2")
                  kO2 = pqkv.tile([128, S], BF16, tag="kO2")
                  for (e, o, de, do) in ((qE, qO, qE2, qO2), (kE, kO, kE2, kO2)):
                      t1 = ptmp.tile([128, S], BF16, tag="rt1")
                      t2 = ptmp.tile([128, S], BF16, tag="rt2")
                      nc.vector.tensor_mul(t1[:], e[:], cos_sb[:])
                      nc.vector.tensor_mul(t2[:], o[:], sin_sb[:])
                      nc.vector.tensor_sub(de[:], t1[:], t2[:])
                      t3 = ptmp.tile([128, S], BF16, tag="rt1")
                      t4 = ptmp.tile([128, S], BF16, tag="rt2")
                      nc.vector.tensor_mul(t3[:], o[:], cos_sb[:])
                      nc.vector.tensor_mul(t4[:], e[:], sin_sb[:])
                      nc.vector.tensor_add(do[:], t3[:], t4[:])

                  # ---- repack to per-head-contiguous q/k tiles ----
                  # qh[t] rows: [head(2t): e(32) o(32), head(2t+1): e(32) o(32)]
                  qh = [pqkv.tile([128, S], BF16, tag=f"qh{t}", name=f"qh{t}") for t in range(2)]
                  kh = [pqkv.tile([128, S], BF16, tag=f"kh{t}", name=f"kh{t}") for t in range(2)]
                  for h in range(4):
                      t, r = divmod(h, 2)
                      nc.vector.tensor_copy(
                          qh[t][64 * r:64 * r + 32, :], qE2[32 * h:32 * (h + 1), :]
                      )
                      nc.vector.tensor_copy(
                          qh[t][64 * r + 32:64 * r + 64, :], qO2[32 * h:32 * (h + 1), :]
                      )
                      nc.vector.tensor_copy(
                          kh[t][64 * r:64 * r + 32, :], kE2[32 * h:32 * (h + 1), :]
                      )
                      nc.vector.tensor_copy(
                          kh[t][64 * r + 32:64 * r + 64, :], kO2[32 * h:32 * (h + 1), :]
                      )

                  # ---- v transpose into v_aug [128 kr, 16*(64+1)] per head ----
                  vaug = []
                  for h in range(4):
                      va = pqkv.tile([128, 16 * 65], BF16, tag=f"vaug{h}", name=f"vaug{h}")
                      nc.vector.memset(va[:], 1.0)  # ones columns survive at 65c+64
                      vaug.append(va)
                  for h in range(4):
                      t, r = divmod(h, 2)
                      vsrc = v0 if t == 0 else v1
                      for c0 in range(0, 16, 4):
                          trp = psB.tile([128, 256], BF16, tag="mmB", name="trp")
                          for u in range(4):
                              nc.tensor.transpose(
                                  trp[:, 64 * u:64 * (u + 1)],
                                  vsrc[64 * r:64 * r + 64,
                                       128 * (c0 + u):128 * (c0 + u + 1)],
                                  ident_sb[64 * r:64 * r + 64, 64 * r:64 * r + 64],
                              )
                          dst = vaug[h][:, 65 * c0:65 * (c0 + 4)].rearrange(
                              "p (c k) -> p c k", c=4
                          )[:, :, 0:64]
                          nc.vector.tensor_copy(
                              dst,
                              trp[:].rearrange("p (c k) -> p c k", c=4),
                          )

                  # ---- collective buffers ----
                  cc_in = pdram.tile([256, S], BF16, tag="ccin", name=f"ccin_{rep}")
                  gat = pdram.tile([1024, S], BF16, tag="gat", name=f"gat_{rep}")

                  # ---- attention (qr-block outer, head inner) + AG + out-proj ----
                  def outproj(j):
                      aggs = []
                      for i in range(8):
                          a = pagg.tile([128, QB], BF16, tag=f"agg{i}", name=f"agg{i}")
                          nc.sync.dma_start(
                              a[:],
                              gat[128 * i:128 * (i + 1), QB * j:QB * (j + 1)],
                          )
                          aggs.append(a)
                      for ot2 in range(2):
                          op = psA.tile([128, QB], FP32, tag="mmA")
                          for i in range(8):
                              nc.tensor.matmul(
                                  op[:],
                                  lhsT=wout_sb[i][:, 128 * ot2:128 * (ot2 + 1)],
                                  rhs=aggs[i][:],
                                  start=(i == 0),
                                  stop=(i == 7),
                              )
                          osb = pout.tile([128, QB], FP32, tag="osb")
                          nc.vector.tensor_copy(osb[:], op[:])
                          for half in range(2):
                              nc.sync.dma_start(
                                  out_e[2 * j + half][
                                      128 * ot2:128 * (ot2 + 1), :
                                  ],
                                  osb[:, 256 * half:256 * (half + 1)],
                              )

                  for qb in range(4):
                      for hp in range(2):
                          h0, h1 = 2 * hp, 2 * hp + 1
                          at0 = psAT.tile([65, QB], FP32, tag="at")
                          at1 = psAT.tile([65, QB], FP32, tag="at")
                          nkc = 4 * qb + 4
                          for c in range(nkc):
                              # both heads' score chunks side by side, one exp
                              scp = psB.tile([128, 2 * QB], FP32, tag="mmB")
                              diag = c >= 4 * qb
                              nc.tensor.matmul(
                                  scp[:, 0:QB],
                                  lhsT=kh[hp][0:64, 128 * c:128 * (c + 1)],
                                  rhs=qh[hp][0:64, QB * qb:QB * (qb + 1)],
                                  start=True,
                                  stop=not diag,
                              )
                              nc.tensor.matmul(
                                  scp[:, QB:2 * QB],
                                  lhsT=kh[hp][64:128, 128 * c:128 * (c + 1)],
                                  rhs=qh[hp][64:128, QB * qb:QB * (qb + 1)],
                                  start=True,
                                  stop=not diag,
                              )
                              if c >= 4 * qb:
                                  # additive -400 penalty on masked elements via
                                  # identity-lhsT matmul; exp then yields 0.0
                                  tloc = c - 4 * qb
                                  for half in range(2):
                                      nc.tensor.matmul(
                                          scp[:, QB * half:QB * (half + 1)],
                                          lhsT=ident_sb[:, :],
                                          rhs=dmask_sb[:, QB * tloc:QB * (tloc + 1)],
                                          start=False,
                                          stop=True,
                                      )
                              wt = pwt.tile([128, 2 * QB], BF16, tag="wt")
                              nc.scalar.activation(wt[:], scp[:], Exp, scale=0.125)
                              nc.tensor.matmul(
                                  at0[:],
                                  lhsT=vaug[h0][:, 65 * c:65 * c + 65],
                                  rhs=wt[:, 0:QB],
                                  start=(c == 0),
                                  stop=(c == nkc - 1),
                              )
                              nc.tensor.matmul(
                                  at1[:],
                                  lhsT=vaug[h1][:, 65 * c:65 * c + 65],
                                  rhs=wt[:, QB:2 * QB],
                                  start=(c == 0),
                                  stop=(c == nkc - 1),
                              )
                          for h, at in ((h0, at0), (h1, at1)):
                              # normalize rows by 1/denom (denom = at row 64)
                              rc = pao.tile([1, QB], FP32, tag="recip")
                              nc.vector.reciprocal(rc[:], at[64:65, :])
                              bc = psA.tile([64, QB], FP32, tag="mmA")
                              nc.tensor.matmul(
                                  bc[:], lhsT=ones_sb[:, :], rhs=rc[:],
                                  start=True, stop=True,
                              )
                              bc_sb = pao.tile([64, QB], FP32, tag="bcsb")
                              nc.vector.tensor_copy(bc_sb[:], bc[:])
                              ao = pao.tile([64, QB], BF16, tag="ao")
                              nc.vector.tensor_mul(ao[:], at[0:64, :], bc_sb[:])
                              nc.sync.dma_start(
                                  cc_in[64 * h:64 * (h + 1),
                                        QB * qb:QB * (qb + 1)],
                                  ao[:],
                              )
                  if collectives:
                      nc.gpsimd.collective_compute(
                          "AllGather",
                          mybir.AluOpType.bypass,
                          replica_groups=RG,
                          ins=[cc_in.opt()],
                          outs=[gat.opt()],
                      )
                  for j in range(4):
                      outproj(j)

    nc.compile()
    return nc


def _prep_core(x, wQKV, wOut, cosT, sinT, dmask, b, g):
    heads = [4 * g + i for i in range(HL)]
    rows = []
    for base in (0, D):          # q rows then k rows
        for par in (0, 1):       # evens then odds
            for h in heads:
                for i in range(NF):
                    rows.append(base + h * DH + 2 * i + par)
    for h in heads:
        for d in range(DH):
            rows.append(2 * D + h * DH + d)
    wqkvT = np.ascontiguousarray(wQKV[rows, :].T).astype(BF)
    woutT = np.ascontiguousarray(wOut[256 * g:256 * (g + 1), :].T).astype(BF)
    xT = np.ascontiguousarray(x[b].T).astype(BF)
    return {
        "xT": xT, "wqkvT": wqkvT, "woutT": woutT,
        "cosT": cosT, "sinT": sinT, "dmask": dmask,
    }


def _log(msg):
    import sys, time
    print(f"[kernel {time.strftime('%H:%M:%S')}] {msg}", file=sys.stderr, flush=True)


def _to_np(v):
    """Convert to host numpy; chunk device fetches (big single d2h transfers
    hang through the axon tunnel)."""
    if isinstance(v, np.ndarray):
        return np.asarray(v, np.float32)
    shape = tuple(v.shape)
    n = int(np.prod(shape))
    if n * 4 <= (1 << 19):
        return np.asarray(v).astype(np.float32)
    flat = v.reshape(-1)
    step = (1 << 19) // 4  # 128K elements = 512KB
    parts = [np.asarray(flat[i:i + step]) for i in range(0, n, step)]
    return np.concatenate(parts).astype(np.float32).reshape(shape)


def kernel(x, wQKV, wOut, cos, sin):
    from concourse.bass_utils import run_bass_kernel_spmd

    x = _to_np(x)
    wQKV = _to_np(wQKV)
    wOut = _to_np(wOut)
    cos = _to_np(cos)
    sin = _to_np(sin)

    cosT = np.ascontiguousarray(np.tile(cos.T[:NF], (4, 1))).astype(BF)  # [128, S]
    sinT = np.ascontiguousarray(np.tile(sin.T[:NF], (4, 1))).astype(BF)
    dm = np.zeros((KC, 4 * QB), np.float32)
    for t in range(4):
        kr = 128 * t + np.arange(KC)[:, None]
        qr = np.arange(QB)[None, :]
        dm[:, QB * t:QB * (t + 1)] = np.where(kr <= qr, 0.0, -400.0)
    dmask = dm.astype(BF)

    if "nc" not in _cached:
        _log("building bass graph...")
        _cached["nc"] = _build_nc()
        _log("graph built")
    nc = _cached["nc"]

    in_maps = []
    for b in range(B):
        for g in range(4):
            in_maps.append(_prep_core(x, wQKV, wOut, cosT, sinT, dmask, b, g))
    _log("in_maps ready; launching run_bass_kernel_spmd (compile+run)...")

    res = run_bass_kernel_spmd(nc, in_maps, core_ids=list(range(NC_)))
    _log("run complete")
    _cached["last_res"] = res
    out = np.zeros((B, S, D), np.float32)
    for b in range(B):
        for g in range(4):
            r = res.results[4 * b + g]
            outT = np.concatenate([r[f"out{j}"] for j in range(8)], axis=1)
            out[b, :, 256 * g:256 * (g + 1)] = outT.T
    return out



# revision 15
# speedup vs baseline: 1.2779x; 1.2779x over previous
"""Distributed Bass kernel for causal multi-head attention with RoPE on 8 TRN2 NeuronCores.

Problem (hardcoded): x [2, 2048, 1024] f32, wQKV [3072, 1024], wOut [1024, 1024],
cos/sin [2048, 32]; 16 heads, dh=64; out = causal-attention(x) @ wOut.T.

Sharding: 8 cores = 2 (batch) x 4 (head-group). Core (b, g) computes heads
4g..4g+3 of batch b. After attention, an AllGather within each 4-core batch
group collects all heads' attn output (transposed layout [dims, seq]); each
core then applies its 256-column slice of wOut, producing outT [256, 2048].
Host reassembles [2, 2048, 1024].

v2 changes vs baseline (215us):
- V^T computed directly from the projection (x-chunk stationary, wV moving)
  instead of V + 64 PE transposes.
- Causal mask: only the 128x128 diagonal triangle is multiplied by a 0/1
  mask on DVE; QK/PV matmuls subrange away fully-masked columns. The
  identity-matmul additive-mask trick (PE) is gone.
- Softmax normalize broadcasts both heads' 1/denom with one [2,128]-selector
  matmul instead of two ones-matmuls.
- AllGather split in two per rep (qb pairs), fired mid-attention; out-proj
  matmuls interleaved into later attention / next rep's projection so the
  collective latency is off the PE critical path.
- Cross-rep software pipelining: rep r's QKV/RoPE/V^T emission is
  interleaved into rep r-1's attention stream (attention is ScalarE-bound,
  projection is TensorE-bound). qh/kh/vaug are double-buffered for this.

Compute dtype: bf16 matmuls with f32 PSUM accumulation.
"""

import numpy as np
import ml_dtypes

BF = ml_dtypes.bfloat16

B, S, D, H = 2, 2048, 1024, 16
DH = 64          # head dim
NF = 32          # rope freqs = DH/2
HL = 4           # heads per core
QB = 512         # qr block width
KC = 128         # kr chunk
NC_ = 8          # cores
RG = [[0, 1, 2, 3], [4, 5, 6, 7]]

_cached = {}


def _build_nc(nrep=1, collectives=True):
    import concourse.bass as bass
    import concourse.bacc as bacc
    import concourse.mybir as mybir
    import concourse.tile as tile

    FP32 = mybir.dt.float32
    BF16 = mybir.dt.bfloat16
    Exp = mybir.ActivationFunctionType.Exp

    nc = bacc.Bacc(
        "TRN2", target_bir_lowering=False, debug=False, num_devices=NC_
    )

    xT_e = nc.dram_tensor("xT", [D, S], BF16, kind="ExternalInput")
    wqkvT_e = nc.dram_tensor("wqkvT", [D, 768], BF16, kind="ExternalInput")
    woutT_e = nc.dram_tensor("woutT", [D, 256], BF16, kind="ExternalInput")
    cosT_e = nc.dram_tensor("cosT", [128, S], BF16, kind="ExternalInput")
    sinT_e = nc.dram_tensor("sinT", [128, S], BF16, kind="ExternalInput")
    # tri mask [p, 2*128]: keep (1.0) where p <= j (j = col % 128), else 0
    bmask_e = nc.dram_tensor("bmask", [128, 256], BF16, kind="ExternalInput")
    # output split into 256KB chunks: big single d2h transfers hang through
    # the axon tunnel. out{j} covers s columns [256j, 256j+256).
    out_e = [
        nc.dram_tensor(f"out{j}", [256, 256], FP32, kind="ExternalOutput")
        for j in range(8)
    ]

    with tile.TileContext(nc) as tc:
        with (
            tc.tile_pool(name="pconst", bufs=1) as pconst,
            tc.tile_pool(name="pw", bufs=1) as pw,
            tc.tile_pool(name="px", bufs=1) as px,
            tc.tile_pool(name="pqkv", bufs=1) as pqkv,
            tc.tile_pool(name="pqh", bufs=1) as pqh,
            tc.tile_pool(name="ptmp", bufs=2) as ptmp,
            tc.tile_pool(name="pwt", bufs=6) as pwt,
            tc.tile_pool(name="pao", bufs=2) as pao,
            tc.tile_pool(name="pagg", bufs=2) as pagg,
            tc.tile_pool(name="pout", bufs=2) as pout,
            tc.tile_pool(name="psB", bufs=2, space="PSUM") as psB,
            tc.tile_pool(name="psAT", bufs=3, space="PSUM") as psAT,
            tc.tile_pool(name="pdram", bufs=1, space="DRAM") as pdram,
        ):
            # ---- constants ----
            cos_sb = pconst.tile([128, S], BF16, tag="cos")
            sin_sb = pconst.tile([128, S], BF16, tag="sin")
            bmask_sb = pconst.tile([128, 256], BF16, tag="bmask")
            ones_sb = pconst.tile([1, 128], BF16, tag="ones")
            nc.sync.dma_start(cos_sb[:], cosT_e[:, :])
            nc.sync.dma_start(sin_sb[:], sinT_e[:, :])
            nc.sync.dma_start(bmask_sb[:], bmask_e[:, :])
            nc.vector.memset(ones_sb[:], 1.0)

            # ---- weights + x ----
            wqkv_sb = []
            wout_sb = []
            x_sb = []
            for i in range(8):
                w = pw.tile([128, 768], BF16, tag=f"wqkv{i}", name=f"wqkv{i}")
                nc.sync.dma_start(w[:], wqkvT_e[128 * i:128 * (i + 1), :])
                wqkv_sb.append(w)
                wo = pw.tile([128, 256], BF16, tag=f"wout{i}", name=f"wout{i}")
                nc.sync.dma_start(wo[:], woutT_e[128 * i:128 * (i + 1), :])
                wout_sb.append(wo)
                xt = px.tile([128, S], BF16, tag=f"x{i}", name=f"x{i}")
                nc.sync.dma_start(xt[:], xT_e[128 * i:128 * (i + 1), :])
                x_sb.append(xt)

            # ---- persistent SBUF tiles ----
            # od-tiles 0..3 of the projection: qE, qO, kE, kO (single-buffered;
            # consumed by RoPE shortly after they're produced)
            qkv_t = [
                pqkv.tile([128, S], BF16, tag=f"qkv{i}", name=f"qkv{i}")
                for i in range(4)
            ]
            qE2 = pqkv.tile([128, S], BF16, tag="qE2")
            qO2 = pqkv.tile([128, S], BF16, tag="qO2")
            kE2 = pqkv.tile([128, S], BF16, tag="kE2")
            kO2 = pqkv.tile([128, S], BF16, tag="kO2")
            # qh/kh/vaug double-buffered: rep r's repack/vT runs while rep
            # r-1's attention still reads the other buffer.
            qh = [[pqh.tile([128, S], BF16, tag=f"qh{v}{t}", name=f"qh{v}{t}")
                   for t in range(2)] for v in range(2)]
            kh = [[pqh.tile([128, S], BF16, tag=f"kh{v}{t}", name=f"kh{v}{t}")
                   for t in range(2)] for v in range(2)]
            # vaug[v]: [128 kr, 4 heads, 16 chunks, 65] — col 64 of each 65-
            # block is the ones column for the softmax denominator.
            vaug = [pqh.tile([128, 4 * 16 * 65], BF16, tag=f"vaug{v}",
                             name=f"vaug{v}") for v in range(2)]
            nc.vector.memset(vaug[0][:], 1.0)
            nc.vector.memset(vaug[1][:], 1.0)

            # =============== emission units ===============

            def unit_od(rep, ot):
                # one q/k od-tile of the projection: qkvT[od, s] [128, 2048]
                def f():
                    for sp in range(2):
                        ps = psB.tile([128, 2 * QB], FP32, tag="mmB")
                        for half in range(2):
                            sc_i = 2 * sp + half
                            for dc in range(8):
                                nc.tensor.matmul(
                                    ps[:, QB * half:QB * (half + 1)],
                                    lhsT=wqkv_sb[dc][:, 128 * ot:128 * (ot + 1)],
                                    rhs=x_sb[dc][:, QB * sc_i:QB * (sc_i + 1)],
                                    start=(dc == 0),
                                    stop=(dc == 7),
                                )
                        nc.vector.tensor_copy(
                            qkv_t[ot][:, 2 * QB * sp:2 * QB * (sp + 1)], ps[:]
                        )
                return f

            def unit_rope(rep, i):
                # RoPE, 2 DVE ops at a time (12 total): E/O layout full-width
                def f():
                    pairs = [
                        (qkv_t[0], qkv_t[1], qE2, qO2),
                        (qkv_t[2], qkv_t[3], kE2, kO2),
                    ]
                    e, o, de, do = pairs[i // 3]
                    j = i % 3
                    if j == 0:
                        t1 = ptmp.tile([128, S], BF16, tag="rt1")
                        nc.vector.tensor_mul(t1[:], e[:], cos_sb[:])
                        t2 = ptmp.tile([128, S], BF16, tag="rt2")
                        nc.vector.tensor_mul(t2[:], o[:], sin_sb[:])
                        f.t1, f.t2 = t1, t2
                    elif j == 1:
                        prev = unit_rope.state
                        nc.vector.tensor_sub(de[:], prev.t1[:], prev.t2[:])
                        t3 = ptmp.tile([128, S], BF16, tag="rt1")
                        nc.vector.tensor_mul(t3[:], o[:], cos_sb[:])
                        f.t3 = t3
                    else:
                        prev = unit_rope.state
                        t4 = ptmp.tile([128, S], BF16, tag="rt2")
                        nc.vector.tensor_mul(t4[:], e[:], sin_sb[:])
                        nc.vector.tensor_add(do[:], prev.t3[:], t4[:])
                    unit_rope.state = f
                return f

            def unit_repack(rep, h):
                # head h of qE2/qO2/kE2/kO2 -> per-head-contiguous qh/kh
                v = rep % 2
                def f():
                    t, r = divmod(h, 2)
                    nc.vector.tensor_copy(
                        qh[v][t][64 * r:64 * r + 32, :], qE2[32 * h:32 * (h + 1), :]
                    )
                    nc.vector.tensor_copy(
                        qh[v][t][64 * r + 32:64 * r + 64, :], qO2[32 * h:32 * (h + 1), :]
                    )
                    nc.vector.tensor_copy(
                        kh[v][t][64 * r:64 * r + 32, :], kE2[32 * h:32 * (h + 1), :]
                    )
                    nc.vector.tensor_copy(
                        kh[v][t][64 * r + 32:64 * r + 64, :], kO2[32 * h:32 * (h + 1), :]
                    )
                return f

            def unit_vt(rep, sc):
                # v^T for seq chunk sc, all 4 heads: [128 s, 256] directly
                # from the projection (x chunk stationary, wV moving)
                v = rep % 2
                def f():
                    ps = psB.tile([128, 2 * QB], FP32, tag="mmB")
                    for dc in range(8):
                        nc.tensor.matmul(
                            ps[:, 0:256],
                            lhsT=x_sb[dc][:, 128 * sc:128 * (sc + 1)],
                            rhs=wqkv_sb[dc][:, 512:768],
                            start=(dc == 0),
                            stop=(dc == 7),
                        )
                    # scatter 4 heads' 64 cols into the 65-strided vaug layout
                    dst = vaug[v][:].rearrange(
                        "p (h c k) -> p h c k", h=4, c=16
                    )[:, :, sc, 0:64]
                    src = ps[:, 0:256].rearrange("p (h k) -> p h k", h=4)
                    nc.vector.tensor_copy(dst, src)
                return f

            # ---- attention ----

            def unit_chunk(rep, qb, hp, c):
                v = rep % 2
                nkc = 4 * qb + 4
                def f():
                    diag = c >= 4 * qb
                    tloc = c - 4 * qb
                    col0 = 128 * tloc if diag else 0
                    h0, h1 = 2 * hp, 2 * hp + 1
                    if c == 0:
                        f.at0 = psAT.tile([65, QB], FP32, tag="at")
                        f.at1 = psAT.tile([65, QB], FP32, tag="at")
                        unit_chunk.at = (f.at0, f.at1)
                    at0, at1 = unit_chunk.at
                    scp = psB.tile([128, 2 * QB], FP32, tag="mmB")
                    nc.tensor.matmul(
                        scp[:, col0:QB],
                        lhsT=kh[v][hp][0:64, 128 * c:128 * (c + 1)],
                        rhs=qh[v][hp][0:64, QB * qb + col0:QB * (qb + 1)],
                        start=True, stop=True,
                    )
                    nc.tensor.matmul(
                        scp[:, QB + col0:2 * QB],
                        lhsT=kh[v][hp][64:128, 128 * c:128 * (c + 1)],
                        rhs=qh[v][hp][64:128, QB * qb + col0:QB * (qb + 1)],
                        start=True, stop=True,
                    )
                    wt = pwt.tile([128, 2 * QB], BF16, tag="wt")
                    nc.scalar.activation(wt[:], scp[:], Exp, scale=0.125)
                    if diag:
                        # zero the upper triangle of the 128x128 diagonal
                        # block for both heads (keep p <= j)
                        wtv = wt[:].rearrange("p (g q) -> p g q", g=2)
                        nc.vector.tensor_mul(
                            wtv[:, :, col0:col0 + 128],
                            wtv[:, :, col0:col0 + 128],
                            bmask_sb[:].rearrange("p (g q) -> p g q", g=2),
                        )
                    vv = vaug[v][:].rearrange("p (h c k) -> p h c k", h=4, c=16)
                    nc.tensor.matmul(
                        at0[:, col0:QB],
                        lhsT=vv[:, h0, c, :],
                        rhs=wt[:, col0:QB],
                        start=(c == 0),
                        stop=(c == nkc - 1),
                        skip_group_check=True,
                    )
                    nc.tensor.matmul(
                        at1[:, col0:QB],
                        lhsT=vv[:, h1, c, :],
                        rhs=wt[:, QB + col0:2 * QB],
                        start=(c == 0),
                        stop=(c == nkc - 1),
                        skip_group_check=True,
                    )
                    if c == nkc - 1:
                        _normalize(rep, qb, hp, at0, at1)
                return f

            def _normalize(rep, qb, hp, at0, at1):
                # ao_h = at_h[0:64] * (1/denom) with denom = at_h row 64;
                # both heads' reciprocal rows broadcast down the partitions
                # via a ones-row matmul (partition broadcast isn't a DVE op).
                ccin, _ = dram_bufs[rep]
                rc = pao.tile([1, 2 * QB], BF16, tag="recip")
                with nc.allow_low_precision(
                    reason="1/denom in bf16: uniform per-query scale, ~2^-9 rel"
                ):
                    nc.vector.reciprocal(rc[0:1, 0:QB], at0[64:65, :])
                    nc.vector.reciprocal(rc[0:1, QB:2 * QB], at1[64:65, :])
                bc = psB.tile([128, 2 * QB], FP32, tag="mmB")
                for i in range(2):
                    nc.tensor.matmul(
                        bc[:, QB * i:QB * (i + 1)],
                        lhsT=ones_sb[:], rhs=rc[0:1, QB * i:QB * (i + 1)],
                        start=True, stop=True,
                    )
                bc_sb = pao.tile([128, 2 * QB], FP32, tag="bcsb")
                nc.vector.tensor_copy(bc_sb[:], bc[:])
                for i, at in ((0, at0), (1, at1)):
                    h = 2 * hp + i
                    ao = pao.tile([64, QB], BF16, tag="ao")
                    nc.vector.tensor_mul(
                        ao[:], at[0:64, :], bc_sb[0:64, QB * i:QB * (i + 1)]
                    )
                    nc.sync.dma_start(
                        ccin[qb // 2][64 * h:64 * (h + 1),
                                      QB * (qb % 2):QB * (qb % 2 + 1)],
                        ao[:],
                    )

            def unit_ag(rep, pair):
                ccin, gat = dram_bufs[rep]
                def f():
                    if collectives:
                        nc.gpsimd.collective_compute(
                            "AllGather",
                            mybir.AluOpType.bypass,
                            replica_groups=RG,
                            ins=[ccin[pair].opt()],
                            outs=[gat[pair].opt()],
                        )
                return f

            def unit_aggload(rep, j):
                _, gat = dram_bufs[rep]
                def f():
                    aggs = []
                    for i in range(8):
                        a = pagg.tile([128, QB], BF16, tag=f"agg{i}",
                                      name=f"agg{i}")
                        nc.sync.dma_start(
                            a[:],
                            gat[j // 2][128 * i:128 * (i + 1),
                                        QB * (j % 2):QB * (j % 2 + 1)],
                        )
                        aggs.append(a)
                    unit_aggload.aggs[(rep, j)] = aggs
                return f
            unit_aggload.aggs = {}

            def unit_outproj(rep, j, ot2):
                def f():
                    aggs = unit_aggload.aggs[(rep, j)]
                    op = psAT.tile([128, QB], FP32, tag="op", bufs=1)
                    for i in range(8):
                        nc.tensor.matmul(
                            op[:],
                            lhsT=wout_sb[i][:, 128 * ot2:128 * (ot2 + 1)],
                            rhs=aggs[i][:],
                            start=(i == 0),
                            stop=(i == 7),
                        )
                    osb = pout.tile([128, QB], FP32, tag="osb")
                    nc.vector.tensor_copy(osb[:], op[:])
                    for half in range(2):
                        nc.sync.dma_start(
                            out_e[2 * j + half][128 * ot2:128 * (ot2 + 1), :],
                            osb[:, 256 * half:256 * (half + 1)],
                        )
                return f

            # =============== schedule ===============

            dram_bufs = []
            for rep in range(nrep):
                ccin = [pdram.tile([256, 2 * QB], BF16, tag=f"ccin{p}",
                                   name=f"ccin_{rep}_{p}") for p in range(2)]
                gat = [pdram.tile([1024, 2 * QB], BF16, tag=f"gat{p}",
                                  name=f"gat_{rep}_{p}") for p in range(2)]
                dram_bufs.append((ccin, gat))

            def units_A(rep):
                us = []
                for ot in range(4):
                    us.append(unit_od(rep, ot))
                for i in range(6):
                    us.append(unit_rope(rep, i))
                # interleave repack and vt so attention inputs for low (qb,hp)
                # are ready earliest
                for h in range(4):
                    us.append(unit_repack(rep, h))
                    us.append(unit_vt(rep, 4 * h + 0))
                    us.append(unit_vt(rep, 4 * h + 1))
                    us.append(unit_vt(rep, 4 * h + 2))
                    us.append(unit_vt(rep, 4 * h + 3))
                return us

            def units_B(rep):
                # attention chunks + AG fires; aggload/outproj units are
                # returned separately (emitted later, once their AG is done)
                us = []
                outs = []
                for qb in range(4):
                    for hp in range(2):
                        for c in range(4 * qb + 4):
                            us.append(unit_chunk(rep, qb, hp, c))
                    if qb == 1 or qb == 3:
                        us.append(unit_ag(rep, qb // 2))
                for j in range(4):
                    outs.append(unit_aggload(rep, j))
                    for ot2 in range(2):
                        outs.append(unit_outproj(rep, j, ot2))
                return us, outs

            def interleave(primary, secondary):
                # emit all of `primary` in order, with `secondary` (also in
                # order) spread evenly through it
                np_, ns = len(primary), len(secondary)
                if np_ == 0:
                    for u in secondary:
                        u()
                    return
                k = 0
                for i, u in enumerate(primary):
                    u()
                    want = (i + 1) * ns // np_
                    while k < want:
                        secondary[k]()
                        k += 1

            def inject(primary, carry, start=24, step=4):
                # splice `carry` units into `primary` one every `step` units
                # beginning at `start` (i.e. during qb1/qb2 of the attention)
                out = list(primary[:start])
                pos = start
                for cu in carry:
                    out.extend(primary[pos:pos + step])
                    out.append(cu)
                    pos += step
                out.extend(primary[pos:])
                return out

            prevB = []
            prev_outs = []
            for rep in range(nrep):
                A = units_A(rep)
                # rep r-1's attention stream carries rep r's projection; rep
                # r-1's outproj for qb 0,1 rides in the back (AG01 fired ~35%
                # into the attention stream, these land past 80%).
                interleave(prevB, A + prev_outs[:6])
                carry = prev_outs[6:]   # outproj qb 2,3: AG23 just fired
                newB, prev_outs = units_B(rep)
                prevB = inject(newB, carry)
            # epilogue: final rep's attention, then its outproj
            interleave(prevB, [])
            for u in prev_outs:
                u()

    nc.compile()
    return nc


def _prep_core(x, wQKV, wOut, cosT, sinT, bmask, b, g):
    heads = [4 * g + i for i in range(HL)]
    rows = []
    for base in (0, D):          # q rows then k rows
        for par in (0, 1):       # evens then odds
            for h in heads:
                for i in range(NF):
                    rows.append(base + h * DH + 2 * i + par)
    for h in heads:
        for d in range(DH):
            rows.append(2 * D + h * DH + d)
    wqkvT = np.ascontiguousarray(wQKV[rows, :].T).astype(BF)
    woutT = np.ascontiguousarray(wOut[256 * g:256 * (g + 1), :].T).astype(BF)
    xT = np.ascontiguousarray(x[b].T).astype(BF)
    return {
        "xT": xT, "wqkvT": wqkvT, "woutT": woutT,
        "cosT": cosT, "sinT": sinT, "bmask": bmask,
    }


def _log(msg):
    import sys, time
    print(f"[kernel {time.strftime('%H:%M:%S')}] {msg}", file=sys.stderr, flush=True)


def _to_np(v):
    """Convert to host numpy; chunk device fetches (big single d2h transfers
    hang through the axon tunnel)."""
    if isinstance(v, np.ndarray):
        return np.asarray(v, np.float32)
    shape = tuple(v.shape)
    n = int(np.prod(shape))
    if n * 4 <= (1 << 19):
        return np.asarray(v).astype(np.float32)
    flat = v.reshape(-1)
    step = (1 << 19) // 4  # 128K elements = 512KB
    parts = [np.asarray(flat[i:i + step]) for i in range(0, n, step)]
    return np.concatenate(parts).astype(np.float32).reshape(shape)


def _host_consts(cos, sin):
    cosT = np.ascontiguousarray(np.tile(cos.T[:NF], (4, 1))).astype(BF)  # [128, S]
    sinT = np.ascontiguousarray(np.tile(sin.T[:NF], (4, 1))).astype(BF)
    p = np.arange(128)[:, None]
    j = np.arange(128)[None, :]
    tri = (p <= j).astype(np.float32)
    bmask = np.concatenate([tri, tri], axis=1).astype(BF)  # [128, 256]
    return cosT, sinT, bmask


def kernel(x, wQKV, wOut, cos, sin):
    from concourse.bass_utils import run_bass_kernel_spmd

    x = _to_np(x)
    wQKV = _to_np(wQKV)
    wOut = _to_np(wOut)
    cos = _to_np(cos)
    sin = _to_np(sin)

    cosT, sinT, bmask = _host_consts(cos, sin)

    if "nc" not in _cached:
        _log("building bass graph...")
        _cached["nc"] = _build_nc()
        _log("graph built")
    nc = _cached["nc"]

    in_maps = []
    for b in range(B):
        for g in range(4):
            in_maps.append(
                _prep_core(x, wQKV, wOut, cosT, sinT, bmask, b, g)
            )
    _log("in_maps ready; launching run_bass_kernel_spmd (compile+run)...")

    res = run_bass_kernel_spmd(nc, in_maps, core_ids=list(range(NC_)))
    _log("run complete")
    _cached["last_res"] = res
    out = np.zeros((B, S, D), np.float32)
    for b in range(B):
        for g in range(4):
            r = res.results[4 * b + g]
            outT = np.concatenate([r[f"out{j}"] for j in range(8)], axis=1)
            out[b, :, 256 * g:256 * (g + 1)] = outT.T
    return out
